# revision 9
# baseline (speedup 1.0000x reference)
"""Trainium2 Bass kernel for nn_Decoder_45784351375646.

Bahdanau-attention LSTM decoder, WINDOW=64, BATCH=2048, E=D=64.
Data-parallel over batch: 8 cores x 256 batch rows each. Weights replicated.

Key restructurings (validated against the jax reference in numpy):
  - context vector never materialized: hp = h_enc @ p_ctx precomputed once;
    the scan only needs sum_w softmax(v_out) * hp.
  - u_out = h_enc @ u_w.T computed on device via PE matmuls from a
    host-transposed h_encoder (layout prep only); u_b + w_b folded into the
    per-step w_out matmul as a rank-1 bias row.
  - p_b folded into the LSTM gate bias; sigmoid expressed via tanh
    (sig(x) = (1+tanh(x/2))/2) so ACT stays on one table set; h,c stored
    doubled (H2=2h, C2=2c) with the 0.5 factors folded into weight constants.
  - softmax without max-subtraction (|v_out| <= sum|v| ~ 2.6, exp-safe).

Layout: batch on partitions (p = b%128, j = b//128 per core), free = (j,w,e).
States HT/CT kept transposed [d, (j,b)] so all matmuls need no transposes;
only y_tilde is transposed per step (PE transpose + 1-row copy).
"""

import numpy as np

W, B, E, D = 64, 2048, 64, 64
NCORES = 8
BC = B // NCORES          # batch per core = 256
FD = 2 * W * E            # 8192 free elems per partition for the big tensors


# ---------------------------------------------------------------------------
# Workaround: this walrus build accepts at most ONE embedded sem wait per
# instruction, but TileContext._drain_and_barrier attaches every outstanding
# proc's wait to a single SP Drain. Split them one-per-Drain (add_sem_waits
# elides procs SP already observed, so most of these drains carry 0 waits).
# ---------------------------------------------------------------------------
def _split_multiwait_bir(d):
    """Hoist extra embedded sem waits (>1 per instruction) onto standalone
    NoOps on the same engine — this walrus build's codegen accepts at most
    one sync wait command per instruction."""
    ctr = [0]

    def fix_block(b):
        out = []
        for inst in b.get("instructions", []):
            si = inst.get("sync_info") or {}
            waits = si.get("on_wait") or []
            if len(waits) > 1:
                eng = inst.get("engine")
                for wcond in waits[:-1]:
                    ctr[0] += 1
                    out.append({
                        "debug": inst.get("debug", 0), "engine": eng,
                        "ins": [], "name": f"WSPLIT-{ctr[0]}", "opcode": "NoOp",
                        "outs": [],
                        "sync_info": {"on_update": [], "on_wait": [wcond]},
                    })
                si["on_wait"] = waits[-1:]
            out.append(inst)
        b["instructions"] = out
        for sb in b.get("blocks", []):
            fix_block(sb)

    for fn in d["functions"]:
        for b in fn.get("instruction_blocks", fn.get("blocks", [])):
            fix_block(b)
    return d


def _install_tile_patch():
    import json as _json
    import concourse.bass as bass_mod
    import concourse.tile as tile_mod
    from concourse.vector_clock import ScopedClock, VectorClock

    if not getattr(bass_mod.Bass, "_multiwait_patched", False):
        _orig_tjb = bass_mod.Bass.to_json_bytes

        def to_json_bytes_patched(self):
            d = _json.loads(_orig_tjb(self))
            _split_multiwait_bir(d)
            return _json.dumps(d).encode()

        bass_mod.Bass.to_json_bytes = to_json_bytes_patched
        bass_mod.Bass._multiwait_patched = True

    def _patched(self, tick_clock, wait_clock):
        vc = tick_clock.global_clock
        n = len(vc)
        for proc in range(n):
            t = vc[proc]
            if t <= 0:
                continue
            single = VectorClock([0] * n)
            single.require_at_least(proc, t)
            d = self.nc.sync.drain()
            wait_clock.add_sem_waits(d.ins, ScopedClock({None: single}))
        self.nc.all_engine_barrier()
        popped = self.nc._tile_sem_poison_stack.pop()
        assert popped is self._sem_poison
        self.nc.clear_and_free_semaphores(list(self.sems.allocated().values()))
        self.nc.all_engine_barrier()

    tile_mod.TileContext._drain_and_barrier = _patched


def _pack_consts(inputs):
    """Host-side packing of the tiny weight tensors into device layouts."""
    f32 = np.float32
    w_ih = np.asarray(inputs["w_ih"], f32)
    w_hh = np.asarray(inputs["w_hh"], f32)
    b_ih = np.asarray(inputs["b_ih"], f32)
    b_hh = np.asarray(inputs["b_hh"], f32)
    v_w = np.asarray(inputs["v_w"], f32)
    w_w = np.asarray(inputs["w_w"], f32)
    w_b = np.asarray(inputs["w_b"], f32)
    u_w = np.asarray(inputs["u_w"], f32)
    u_b = np.asarray(inputs["u_b"], f32)
    p_w = np.asarray(inputs["p_w"], f32)
    p_b = np.asarray(inputs["p_b"], f32)
    y_w = np.asarray(inputs["y_w"], f32)
    y_b = np.asarray(inputs["y_b"], f32)

    p_ctx = p_w[0, 1:]
    c = {}
    c["p_w0"] = float(p_w[0, 0])
    c["y_b0"] = float(y_b[0])
    # u_out matmul rhs: [E', E+1], last col = p_ctx (gives hp for free)
    c["uwb"] = np.ascontiguousarray(
        np.concatenate([u_w.T, p_ctx[:, None]], axis=1), dtype=f32
    )
    # w_out matmuls (states are doubled -> 0.5 fold); bias row = u_b + w_b
    wwhub = np.zeros((D + 1, E), f32)
    wwhub[:D, :] = 0.5 * w_w[:, :D].T
    wwhub[D, :] = u_b + w_b
    c["wwhub"] = wwhub
    c["wwc"] = np.ascontiguousarray(0.5 * w_w[:, D:].T, dtype=f32)
    # gates (transposed), order i,f,g,o along the packed axis
    bias_all = b_ih + b_hh + p_b[0] * w_ih[:, 0]
    whht = np.zeros((D, 4 * D), f32)
    w2 = np.zeros((2, 4 * D), f32)
    for gi in range(4):
        sl = slice(gi * D, (gi + 1) * D)
        whht[:, sl] = 0.5 * w_hh[sl, :].T
        w2[0, sl] = w_ih[sl, 0]
        w2[1, sl] = bias_all[sl]
    c["whht"] = whht
    c["w2"] = np.ascontiguousarray(w2)
    # dense v tile for the GPSIMD multiply: [128, 32*E]
    c["vb"] = np.ascontiguousarray(np.tile(v_w[0], (128, 32)), dtype=f32)
    # final projection [D, 2] (0.5 fold for doubled states)
    yw = np.zeros((D, 2), f32)
    yw[:, 0] = 0.5 * y_w[0, :D]
    yw[:, 1] = 0.5 * y_w[0, D:]
    c["yw"] = yw
    c["idn"] = np.eye(128, dtype=f32)
    c["ones128"] = np.ones((1, 128), f32)
    return c


def _build_program(consts, loop_reps=1):
    import concourse.bass as bass
    import concourse.mybir as mybir
    from concourse.tile import TileContext

    f32 = mybir.dt.float32
    ALU = mybir.AluOpType
    ACTF = mybir.ActivationFunctionType

    nc = bass.Bass("TRN2", target_bir_lowering=False, debug=False)

    het_d = nc.declare_dram_parameter("het", [E, 2 * W * 128], f32, isOutput=False)
    yv_d = nc.declare_dram_parameter("yv", [128, 2 * W], f32, isOutput=False)
    uwb_d = nc.declare_dram_parameter("uwb", [E, E + 1], f32, isOutput=False)
    wwhub_d = nc.declare_dram_parameter("wwhub", [D + 1, E], f32, isOutput=False)
    wwc_d = nc.declare_dram_parameter("wwc", [D, E], f32, isOutput=False)
    whht_d = nc.declare_dram_parameter("whht", [D, 4 * D], f32, isOutput=False)
    w2_d = nc.declare_dram_parameter("w2", [2, 4 * D], f32, isOutput=False)
    vb_d = nc.declare_dram_parameter("vb", [128, 32 * E], f32, isOutput=False)
    yw_d = nc.declare_dram_parameter("yw", [D, 2], f32, isOutput=False)
    idn_d = nc.declare_dram_parameter("idn", [128, 128], f32, isOutput=False)
    ones_d = nc.declare_dram_parameter("ones128", [1, 128], f32, isOutput=False)
    out_d = nc.declare_dram_parameter("out", [1, BC], f32, isOutput=True)

    with TileContext(nc) as tc:
        with tc.tile_pool(name="main", bufs=1) as mp, \
             tc.tile_pool(name="scr", bufs=2) as sp, \
             tc.tile_pool(name="psum", bufs=1, space="PSUM") as pp:

            # ---- persistent tiles ----
            U = mp.tile([128, FD], f32, tag="U")
            HP = mp.tile([128, 2 * W], f32, tag="HP")
            YV = mp.tile([128, 2 * W], f32, tag="YV")
            VB = mp.tile([128, 32 * E], f32, tag="VB")
            UWB = mp.tile([E, E + 1], f32, tag="UWB")
            WWHUB = mp.tile([D + 1, E], f32, tag="WWHUB")
            WWC = mp.tile([D, E], f32, tag="WWC")
            WHHT = mp.tile([D, 4 * D], f32, tag="WHHT")
            W2A = mp.tile([2, 4 * D], f32, tag="W2A")
            YW = mp.tile([D, 2], f32, tag="YW")
            IDN = mp.tile([128, 128], f32, tag="IDN")
            HT65 = mp.tile([D + 1, 2 * 128], f32, tag="HT65")   # rows 0-63 H2, row 64 ones
            CT = mp.tile([D, 2 * 128], f32, tag="CT")           # C2
            Y2 = [
                mp.tile([2, 128], f32, tag=f"Y2_{j}", name=f"Y2_{j}")
                for j in range(2)
            ]

            # ---- const loads + state init ----
            nc.sync.dma_start(YV[:, :], yv_d[:, :])
            nc.sync.dma_start(VB[:, :], vb_d[:, :])
            nc.sync.dma_start(UWB[:, :], uwb_d[:, :])
            nc.sync.dma_start(WWHUB[:, :], wwhub_d[:, :])
            nc.sync.dma_start(WWC[:, :], wwc_d[:, :])
            nc.sync.dma_start(WHHT[:, :], whht_d[:, :])
            nc.sync.dma_start(W2A[:, :], w2_d[:, :])
            nc.sync.dma_start(YW[:, :], yw_d[:, :])
            nc.sync.dma_start(IDN[:, :], idn_d[:, :])
            nc.vector.memset(HT65[0:D, :], 0.0)
            nc.vector.memset(HT65[D:D + 1, :], 1.0)
            nc.vector.memset(CT[:, :], 0.0)
            for j in range(2):
                nc.sync.dma_start(Y2[j][1:2, :], ones_d[:, :])

            # ---- precompute U (u_out w/o bias) and HP ----
            with tc.tile_pool(name="prep", bufs=1) as prep:
                HET = prep.tile([E, 2 * W * 128], f32, tag="HET")
                nc.sync.dma_start(HET[:, :], het_d[:, :])
                for j in range(2):
                    for wg in range(W // 4):
                        PU = pp.tile([128, 4 * (E + 1)], f32, tag="PG0",
                                     name="PU")
                        for k in range(4):
                            w = wg * 4 + k
                            nc.tensor.matmul(
                                PU[:, k * (E + 1):(k + 1) * (E + 1)],
                                HET[:, j * W * 128 + w * 128:j * W * 128 + (w + 1) * 128],
                                UWB[:, :],
                                start=True, stop=True,
                            )
                        puv = PU[:, :].rearrange("p (k x) -> p k x", x=E + 1)
                        nc.vector.tensor_copy(
                            U[:, j * W * E + wg * 4 * E:j * W * E + (wg + 1) * 4 * E]
                            .rearrange("p (k e) -> p k e", e=E),
                            puv[:, :, 0:E],
                        )
                        nc.scalar.copy(
                            HP[:, j * W + wg * 4:j * W + (wg + 1) * 4],
                            puv[:, :, E],
                        )

            # ---- the 63-step recurrence ----
            for i in list(range(1, W)) * loop_reps:
                WO = pp.tile([128, 2 * E], f32, tag="WO")
                for j in range(2):
                    nc.tensor.matmul(
                        WO[:, j * E:(j + 1) * E],
                        HT65[:, j * 128:(j + 1) * 128], WWHUB[:, :],
                        start=True, stop=False,
                    )
                    nc.tensor.matmul(
                        WO[:, j * E:(j + 1) * E],
                        CT[:, j * 128:(j + 1) * 128], WWC[:, :],
                        start=False, stop=True,
                    )

                VOUT = sp.tile([128, 2 * W], f32, tag="VOUT")
                CH = FD // 4
                for cch in range(4):
                    j = cch // 2
                    sl = slice(cch * CH, (cch + 1) * CH)
                    Tc = sp.tile([128, CH], f32, tag=f"Tt{cch}", name=f"Tt{cch}",
                                 bufs=1)
                    wo_b = (
                        WO[:, j * E:(j + 1) * E]
                        .unsqueeze(1)
                        .broadcast_to([128, 32, E])
                    )
                    nc.vector.tensor_tensor(
                        Tc[:, :].rearrange("p (w e) -> p w e", e=E),
                        U[:, sl].rearrange("p (w e) -> p w e", e=E),
                        wo_b, op=ALU.add,
                    )
                    nc.scalar.activation(Tc[:, :], Tc[:, :], ACTF.Tanh)
                    nc.gpsimd.tensor_tensor(Tc[:, :], Tc[:, :], VB[:, :], op=ALU.mult)
                    nc.vector.tensor_reduce(
                        VOUT[:, cch * 32:(cch + 1) * 32],
                        Tc[:, :].rearrange("p (w e) -> p w e", e=E),
                        axis=mybir.AxisListType.X, op=ALU.add,
                    )

                EXS = sp.tile([128, 2 * W], f32, tag="EXS")
                SCR = sp.tile([128, 2 * W], f32, tag="SCR")
                DEN = sp.tile([128, 2], f32, tag="DEN")
                NUM = sp.tile([128, 2], f32, tag="NUM")
                RD = sp.tile([128, 2], f32, tag="RD")
                S = sp.tile([128, 2], f32, tag="S")
                YT = sp.tile([128, 2], f32, tag="YT")
                for j in range(2):
                    nc.scalar.activation(
                        EXS[:, j * W:(j + 1) * W], VOUT[:, j * W:(j + 1) * W],
                        ACTF.Exp, accum_out=DEN[:, j:j + 1],
                    )
                    nc.vector.scalar_tensor_tensor(
                        SCR[:, j * W:(j + 1) * W],
                        EXS[:, j * W:(j + 1) * W], 1.0, HP[:, j * W:(j + 1) * W],
                        op0=ALU.mult, op1=ALU.mult, accum_out=NUM[:, j:j + 1],
                    )
                nc.vector.reciprocal(RD[:, :], DEN[:, :])
                nc.vector.tensor_tensor(S[:, :], NUM[:, :], RD[:, :], op=ALU.mult)
                yv_sl = YV[:, :].rearrange("p (j i) -> p j i", j=2)[:, :, i - 1]
                nc.vector.scalar_tensor_tensor(
                    YT[:, :], yv_sl, consts["p_w0"], S[:, :],
                    op0=ALU.mult, op1=ALU.add,
                )
                for j in range(2):
                    YTP = pp.tile([1, 128], f32, tag=f"YTP{j}", name=f"YTP{j}")
                    nc.tensor.transpose(YTP[0:1, :], YT[:, j:j + 1], IDN[:, :])
                    nc.vector.tensor_copy(Y2[j][0:1, :], YTP[0:1, :])

                PGs = []
                for gi in range(4):
                    PG = pp.tile([D, 256], f32, tag=f"PG{gi}")
                    PGs.append(PG)
                    nc.tensor.matmul(
                        PG[:, :], WHHT[:, gi * D:(gi + 1) * D], HT65[0:D, :],
                        start=True, stop=False,
                    )
                    for j in range(2):
                        nc.tensor.matmul(
                            PG[:, j * 128:(j + 1) * 128],
                            W2A[:, gi * D:(gi + 1) * D], Y2[j][:, :],
                            start=False, stop=(j == 1),
                        )

                TI = sp.tile([D, 256], f32, tag="TI")
                TF = sp.tile([D, 256], f32, tag="TF")
                TG = sp.tile([D, 256], f32, tag="TG")
                TO = sp.tile([D, 256], f32, tag="TO")
                nc.scalar.activation(TI[:, :], PGs[0][:, :], ACTF.Tanh, scale=0.5)
                nc.scalar.activation(TF[:, :], PGs[1][:, :], ACTF.Tanh, scale=0.5)
                nc.scalar.activation(TG[:, :], PGs[2][:, :], ACTF.Tanh)
                nc.scalar.activation(TO[:, :], PGs[3][:, :], ACTF.Tanh, scale=0.5)

                A1 = sp.tile([D, 256], f32, tag="A1")
                A2 = sp.tile([D, 256], f32, tag="A2")
                TC = sp.tile([D, 256], f32, tag="TC")
                nc.vector.scalar_tensor_tensor(
                    A1[:, :], TF[:, :], 1.0, CT[:, :], op0=ALU.add, op1=ALU.mult)
                nc.vector.scalar_tensor_tensor(
                    A2[:, :], TI[:, :], 1.0, TG[:, :], op0=ALU.add, op1=ALU.mult)
                nc.vector.scalar_tensor_tensor(
                    CT[:, :], A1[:, :], 0.5, A2[:, :], op0=ALU.mult, op1=ALU.add)
                nc.scalar.activation(TC[:, :], CT[:, :], ACTF.Tanh, scale=0.5)
                nc.vector.scalar_tensor_tensor(
                    HT65[0:D, :], TO[:, :], 1.0, TC[:, :], op0=ALU.add, op1=ALU.mult)

            # ---- final projection ----
            FIN = pp.tile([1, BC], f32, tag="WO", name="FIN")
            nc.tensor.matmul(FIN[:, :], YW[:, 0:1], HT65[0:D, :], start=True, stop=False)
            nc.tensor.matmul(FIN[:, :], YW[:, 1:2], CT[:, :], start=False, stop=True)
            OUTR = mp.tile([1, BC], f32, tag="OUTR")
            nc.vector.tensor_scalar_add(OUTR[:, :], FIN[:, :], consts["y_b0"])
            nc.sync.dma_start(out_d[:, :], OUTR[:, :])

    return nc


consts = None  # set inside kernel(); _build_program reads p_w0/y_b0 immediates
TRACE = False       # test harness sets True to capture an NTFF profile
LOOP_REPS = 1       # test harness uses 0/1/2 for differential timing
LAST_RESULT = None  # BassKernelResults of the most recent kernel() call


def kernel(**inputs) -> np.ndarray:
    global consts
    _install_tile_patch()
    from concourse.bass_utils import run_bass_kernel_spmd

    f32 = np.float32
    h_encoder = np.asarray(inputs["h_encoder"], f32)   # (W,B,E)
    y = np.asarray(inputs["y"], f32)                   # (B,W)
    consts = _pack_consts(inputs)

    # host layout prep (no arithmetic): h_enc^T and per-core shards
    ht = np.ascontiguousarray(h_encoder.transpose(2, 0, 1))   # (E,W,B)

    in_maps = []
    for c in range(NCORES):
        slab = ht[:, :, c * BC:(c + 1) * BC]                  # (E,W,256)
        het = np.ascontiguousarray(
            slab.reshape(E, W, 2, 128).transpose(0, 2, 1, 3).reshape(E, 2 * W * 128)
        )
        ys = y[c * BC:(c + 1) * BC, :]                        # (256,W)
        yv = np.ascontiguousarray(
            ys.reshape(2, 128, W).transpose(1, 0, 2).reshape(128, 2 * W)
        )
        in_maps.append({
            "het": het, "yv": yv,
            "uwb": consts["uwb"], "wwhub": consts["wwhub"], "wwc": consts["wwc"],
            "whht": consts["whht"], "w2": consts["w2"], "vb": consts["vb"],
            "yw": consts["yw"], "idn": consts["idn"],
            "ones128": consts["ones128"],
        })

    nc = _build_program(consts, loop_reps=LOOP_REPS)
    global LAST_RESULT
    res = run_bass_kernel_spmd(
        nc, in_maps, core_ids=list(range(NCORES)), trace=TRACE
    )
    LAST_RESULT = res
    out = np.concatenate([res.results[c]["out"].reshape(BC) for c in range(NCORES)])
    return out.astype(np.float32)


if __name__ == "__main__":
    import reference as refmod
    ins = {k: np.asarray(v) for k, v in refmod.setup_inputs().items()}
    got = kernel(**ins)
    import jax.numpy as jnp
    exp = np.asarray(refmod.reference(**{k: jnp.asarray(v) for k, v in ins.items()}))
    rel = np.linalg.norm(got - exp) / np.linalg.norm(exp)
    print("rel l2 err:", rel)


# revision 10
# speedup vs baseline: 316.3090x; 316.3090x over previous
"""Trainium2 Bass kernel for nn_Decoder_45784351375646.

Bahdanau-attention LSTM decoder, WINDOW=64, BATCH=2048, E=D=64.
Data-parallel over batch: 8 cores x 256 batch rows each. Weights replicated.

Key restructurings (validated against the jax reference in numpy):
  - context vector never materialized: hp = h_enc @ p_ctx precomputed once;
    the scan only needs sum_w softmax(v_out) * hp.
  - u_out = h_enc @ u_w.T computed on device via PE matmuls from a
    host-transposed h_encoder (layout prep only); u_b + w_b folded into the
    per-step w_out matmul as a rank-1 bias row.
  - p_b folded into the LSTM gate bias; sigmoid expressed via tanh
    (sig(x) = (1+tanh(x/2))/2) so ACT stays on one table set; h,c stored
    doubled (H2=2h, C2=2c) with the 0.5 factors folded into weight constants.
  - softmax without max-subtraction (|v_out| <= sum|v| ~ 2.6, exp-safe).

Layout: batch on partitions (p = b%128, j = b//128 per core), free = (j,w,e).
States HT/CT kept transposed [d, (j,b)] so all matmuls need no transposes;
only y_tilde is transposed per step (PE transpose + 1-row copy).
"""

import numpy as np

W, B, E, D = 64, 2048, 64, 64
NCORES = 8
BC = B // NCORES          # batch per core = 256
FD = 2 * W * E            # 8192 free elems per partition for the big tensors


# ---------------------------------------------------------------------------
# Workaround: this walrus build accepts at most ONE embedded sem wait per
# instruction, but TileContext._drain_and_barrier attaches every outstanding
# proc's wait to a single SP Drain. Split them one-per-Drain (add_sem_waits
# elides procs SP already observed, so most of these drains carry 0 waits).
# ---------------------------------------------------------------------------
def _split_multiwait_bir(d):
    """Hoist extra embedded sem waits (>1 per instruction) onto standalone
    NoOps on the same engine — this walrus build's codegen accepts at most
    one sync wait command per instruction."""
    ctr = [0]

    def fix_block(b):
        out = []
        for inst in b.get("instructions", []):
            si = inst.get("sync_info") or {}
            waits = si.get("on_wait") or []
            if len(waits) > 1:
                eng = inst.get("engine")
                for wcond in waits[:-1]:
                    ctr[0] += 1
                    out.append({
                        "debug": inst.get("debug", 0), "engine": eng,
                        "ins": [], "name": f"WSPLIT-{ctr[0]}", "opcode": "NoOp",
                        "outs": [],
                        "sync_info": {"on_update": [], "on_wait": [wcond]},
                    })
                si["on_wait"] = waits[-1:]
            out.append(inst)
        b["instructions"] = out
        for sb in b.get("blocks", []):
            fix_block(sb)

    for fn in d["functions"]:
        for b in fn.get("instruction_blocks", fn.get("blocks", [])):
            fix_block(b)
    return d


def _install_tile_patch():
    import json as _json
    import concourse.bass as bass_mod
    import concourse.tile as tile_mod
    from concourse.vector_clock import ScopedClock, VectorClock

    if not getattr(bass_mod.Bass, "_multiwait_patched", False):
        _orig_tjb = bass_mod.Bass.to_json_bytes

        def to_json_bytes_patched(self):
            d = _json.loads(_orig_tjb(self))
            _split_multiwait_bir(d)
            return _json.dumps(d).encode()

        bass_mod.Bass.to_json_bytes = to_json_bytes_patched
        bass_mod.Bass._multiwait_patched = True

    def _patched(self, tick_clock, wait_clock):
        vc = tick_clock.global_clock
        n = len(vc)
        for proc in range(n):
            t = vc[proc]
            if t <= 0:
                continue
            single = VectorClock([0] * n)
            single.require_at_least(proc, t)
            d = self.nc.sync.drain()
            wait_clock.add_sem_waits(d.ins, ScopedClock({None: single}))
        self.nc.all_engine_barrier()
        popped = self.nc._tile_sem_poison_stack.pop()
        assert popped is self._sem_poison
        self.nc.clear_and_free_semaphores(list(self.sems.allocated().values()))
        self.nc.all_engine_barrier()

    tile_mod.TileContext._drain_and_barrier = _patched


def _pack_consts(inputs):
    """Host-side packing of the tiny weight tensors into device layouts."""
    f32 = np.float32
    w_ih = np.asarray(inputs["w_ih"], f32)
    w_hh = np.asarray(inputs["w_hh"], f32)
    b_ih = np.asarray(inputs["b_ih"], f32)
    b_hh = np.asarray(inputs["b_hh"], f32)
    v_w = np.asarray(inputs["v_w"], f32)
    w_w = np.asarray(inputs["w_w"], f32)
    w_b = np.asarray(inputs["w_b"], f32)
    u_w = np.asarray(inputs["u_w"], f32)
    u_b = np.asarray(inputs["u_b"], f32)
    p_w = np.asarray(inputs["p_w"], f32)
    p_b = np.asarray(inputs["p_b"], f32)
    y_w = np.asarray(inputs["y_w"], f32)
    y_b = np.asarray(inputs["y_b"], f32)

    p_ctx = p_w[0, 1:]
    c = {}
    c["p_w0"] = float(p_w[0, 0])
    c["y_b0"] = float(y_b[0])
    # u_out matmul rhs: [E', E+1], last col = p_ctx (gives hp for free)
    c["uwb"] = np.ascontiguousarray(
        np.concatenate([u_w.T, p_ctx[:, None]], axis=1), dtype=f32
    )
    # w_out matmuls (states are doubled -> 0.5 fold); bias row = u_b + w_b
    wwhub = np.zeros((D + 1, E), f32)
    wwhub[:D, :] = 0.5 * w_w[:, :D].T
    wwhub[D, :] = u_b + w_b
    c["wwhub"] = wwhub
    c["wwc"] = np.ascontiguousarray(0.5 * w_w[:, D:].T, dtype=f32)
    # gates (transposed), order i,f,g,o along the packed axis
    bias_all = b_ih + b_hh + p_b[0] * w_ih[:, 0]
    whht = np.zeros((D, 4 * D), f32)
    w2 = np.zeros((2, 4 * D), f32)
    for gi in range(4):
        sl = slice(gi * D, (gi + 1) * D)
        whht[:, sl] = 0.5 * w_hh[sl, :].T
        w2[0, sl] = w_ih[sl, 0]
        w2[1, sl] = bias_all[sl]
    c["whht"] = whht
    c["w2"] = np.ascontiguousarray(w2)
    # dense v tile for the GPSIMD multiply: [128, 32*E]
    c["vb"] = np.ascontiguousarray(np.tile(v_w[0], (128, 32)), dtype=f32)
    # final projection [D, 2] (0.5 fold for doubled states)
    yw = np.zeros((D, 2), f32)
    yw[:, 0] = 0.5 * y_w[0, :D]
    yw[:, 1] = 0.5 * y_w[0, D:]
    c["yw"] = yw
    c["idn"] = np.eye(128, dtype=f32)
    c["ones128"] = np.ones((1, 128), f32)
    return c


def _build_program(consts, loop_reps=1):
    import concourse.bass as bass
    import concourse.mybir as mybir
    from concourse.tile import TileContext

    f32 = mybir.dt.float32
    ALU = mybir.AluOpType
    ACTF = mybir.ActivationFunctionType

    nc = bass.Bass("TRN2", target_bir_lowering=False, debug=False)

    het_d = nc.declare_dram_parameter("het", [E, 2 * W * 128], f32, isOutput=False)
    yv_d = nc.declare_dram_parameter("yv", [128, 2 * W], f32, isOutput=False)
    uwb_d = nc.declare_dram_parameter("uwb", [E, E + 1], f32, isOutput=False)
    wwhub_d = nc.declare_dram_parameter("wwhub", [D + 1, E], f32, isOutput=False)
    wwc_d = nc.declare_dram_parameter("wwc", [D, E], f32, isOutput=False)
    whht_d = nc.declare_dram_parameter("whht", [D, 4 * D], f32, isOutput=False)
    w2_d = nc.declare_dram_parameter("w2", [2, 4 * D], f32, isOutput=False)
    vb_d = nc.declare_dram_parameter("vb", [128, 32 * E], f32, isOutput=False)
    yw_d = nc.declare_dram_parameter("yw", [D, 2], f32, isOutput=False)
    idn_d = nc.declare_dram_parameter("idn", [128, 128], f32, isOutput=False)
    ones_d = nc.declare_dram_parameter("ones128", [1, 128], f32, isOutput=False)
    out_d = nc.declare_dram_parameter("out", [1, BC], f32, isOutput=True)

    with TileContext(nc) as tc:
        with tc.tile_pool(name="main", bufs=1) as mp, \
             tc.tile_pool(name="scr", bufs=2) as sp, \
             tc.tile_pool(name="psum", bufs=1, space="PSUM") as pp:

            # ---- persistent tiles ----
            U = mp.tile([128, FD], f32, tag="U")
            HP = mp.tile([128, 2 * W], f32, tag="HP")
            YV = mp.tile([128, 2 * W], f32, tag="YV")
            VB = mp.tile([128, 32 * E], f32, tag="VB")
            UWB = mp.tile([E, E + 1], f32, tag="UWB")
            WWHUB = mp.tile([D + 1, E], f32, tag="WWHUB")
            WWC = mp.tile([D, E], f32, tag="WWC")
            WHHT = mp.tile([D, 4 * D], f32, tag="WHHT")
            W2A = mp.tile([2, 4 * D], f32, tag="W2A")
            YW = mp.tile([D, 2], f32, tag="YW")
            IDN = mp.tile([128, 128], f32, tag="IDN")
            HT65 = mp.tile([D + 1, 2 * 128], f32, tag="HT65")   # rows 0-63 H2, row 64 ones
            CT = mp.tile([D, 2 * 128], f32, tag="CT")           # C2
            Y2 = [
                mp.tile([2, 128], f32, tag=f"Y2_{j}", name=f"Y2_{j}")
                for j in range(2)
            ]

            # ---- const loads + state init ----
            nc.sync.dma_start(YV[:, :], yv_d[:, :])
            nc.sync.dma_start(VB[:, :], vb_d[:, :])
            nc.sync.dma_start(UWB[:, :], uwb_d[:, :])
            nc.sync.dma_start(WWHUB[:, :], wwhub_d[:, :])
            nc.sync.dma_start(WWC[:, :], wwc_d[:, :])
            nc.sync.dma_start(WHHT[:, :], whht_d[:, :])
            nc.sync.dma_start(W2A[:, :], w2_d[:, :])
            nc.sync.dma_start(YW[:, :], yw_d[:, :])
            nc.sync.dma_start(IDN[:, :], idn_d[:, :])
            nc.vector.memset(HT65[0:D, :], 0.0)
            nc.vector.memset(HT65[D:D + 1, :], 1.0)
            nc.vector.memset(CT[:, :], 0.0)
            for j in range(2):
                nc.sync.dma_start(Y2[j][1:2, :], ones_d[:, :])

            # ---- precompute U (u_out w/o bias) and HP ----
            with tc.tile_pool(name="prep", bufs=1) as prep:
                HET = prep.tile([E, 2 * W * 128], f32, tag="HET")
                nc.sync.dma_start(HET[:, :], het_d[:, :])
                for j in range(2):
                    for wg in range(W // 4):
                        PU = pp.tile([128, 4 * (E + 1)], f32, tag="PG0",
                                     name="PU")
                        for k in range(4):
                            w = wg * 4 + k
                            nc.tensor.matmul(
                                PU[:, k * (E + 1):(k + 1) * (E + 1)],
                                HET[:, j * W * 128 + w * 128:j * W * 128 + (w + 1) * 128],
                                UWB[:, :],
                                start=True, stop=True,
                            )
                        puv = PU[:, :].rearrange("p (k x) -> p k x", x=E + 1)
                        nc.vector.tensor_copy(
                            U[:, j * W * E + wg * 4 * E:j * W * E + (wg + 1) * 4 * E]
                            .rearrange("p (k e) -> p k e", e=E),
                            puv[:, :, 0:E],
                        )
                        nc.scalar.copy(
                            HP[:, j * W + wg * 4:j * W + (wg + 1) * 4],
                            puv[:, :, E],
                        )

            # ---- the 63-step recurrence ----
            for i in list(range(1, W)) * loop_reps:
                WO = pp.tile([128, 2 * E], f32, tag="WO")
                for j in range(2):
                    nc.tensor.matmul(
                        WO[:, j * E:(j + 1) * E],
                        HT65[:, j * 128:(j + 1) * 128], WWHUB[:, :],
                        start=True, stop=False,
                    )
                    nc.tensor.matmul(
                        WO[:, j * E:(j + 1) * E],
                        CT[:, j * 128:(j + 1) * 128], WWC[:, :],
                        start=False, stop=True,
                    )

                VOUT = sp.tile([128, 2 * W], f32, tag="VOUT")
                CH = FD // 4
                for cch in range(4):
                    j = cch // 2
                    sl = slice(cch * CH, (cch + 1) * CH)
                    Tc = sp.tile([128, CH], f32, tag=f"Tt{cch}", name=f"Tt{cch}",
                                 bufs=1)
                    wo_b = (
                        WO[:, j * E:(j + 1) * E]
                        .unsqueeze(1)
                        .broadcast_to([128, 32, E])
                    )
                    nc.vector.tensor_tensor(
                        Tc[:, :].rearrange("p (w e) -> p w e", e=E),
                        U[:, sl].rearrange("p (w e) -> p w e", e=E),
                        wo_b, op=ALU.add,
                    )
                    nc.scalar.activation(Tc[:, :], Tc[:, :], ACTF.Tanh)
                    nc.gpsimd.tensor_tensor(Tc[:, :], Tc[:, :], VB[:, :], op=ALU.mult)
                    nc.vector.tensor_reduce(
                        VOUT[:, cch * 32:(cch + 1) * 32],
                        Tc[:, :].rearrange("p (w e) -> p w e", e=E),
                        axis=mybir.AxisListType.X, op=ALU.add,
                    )

                EXS = sp.tile([128, 2 * W], f32, tag="EXS")
                SCR = sp.tile([128, 2 * W], f32, tag="SCR")
                DEN = sp.tile([128, 2], f32, tag="DEN")
                NUM = sp.tile([128, 2], f32, tag="NUM")
                RD = sp.tile([128, 2], f32, tag="RD")
                S = sp.tile([128, 2], f32, tag="S")
                YT = sp.tile([128, 2], f32, tag="YT")
                for j in range(2):
                    nc.scalar.activation(
                        EXS[:, j * W:(j + 1) * W], VOUT[:, j * W:(j + 1) * W],
                        ACTF.Exp, accum_out=DEN[:, j:j + 1],
                    )
                    nc.vector.scalar_tensor_tensor(
                        SCR[:, j * W:(j + 1) * W],
                        EXS[:, j * W:(j + 1) * W], 1.0, HP[:, j * W:(j + 1) * W],
                        op0=ALU.mult, op1=ALU.mult, accum_out=NUM[:, j:j + 1],
                    )
                nc.vector.reciprocal(RD[:, :], DEN[:, :])
                nc.vector.tensor_tensor(S[:, :], NUM[:, :], RD[:, :], op=ALU.mult)
                yv_sl = YV[:, :].rearrange("p (j i) -> p j i", j=2)[:, :, i - 1]
                nc.vector.scalar_tensor_tensor(
                    YT[:, :], yv_sl, consts["p_w0"], S[:, :],
                    op0=ALU.mult, op1=ALU.add,
                )
                for j in range(2):
                    YTP = pp.tile([1, 128], f32, tag=f"YTP{j}", name=f"YTP{j}")
                    nc.tensor.transpose(YTP[0:1, :], YT[:, j:j + 1], IDN[:, :])
                    nc.vector.tensor_copy(Y2[j][0:1, :], YTP[0:1, :])

                PGs = []
                for gi in range(4):
                    PG = pp.tile([D, 256], f32, tag=f"PG{gi}")
                    PGs.append(PG)
                    nc.tensor.matmul(
                        PG[:, :], WHHT[:, gi * D:(gi + 1) * D], HT65[0:D, :],
                        start=True, stop=False,
                    )
                    for j in range(2):
                        nc.tensor.matmul(
                            PG[:, j * 128:(j + 1) * 128],
                            W2A[:, gi * D:(gi + 1) * D], Y2[j][:, :],
                            start=False, stop=(j == 1),
                        )

                TI = sp.tile([D, 256], f32, tag="TI")
                TF = sp.tile([D, 256], f32, tag="TF")
                TG = sp.tile([D, 256], f32, tag="TG")
                TO = sp.tile([D, 256], f32, tag="TO")
                nc.scalar.activation(TI[:, :], PGs[0][:, :], ACTF.Tanh, scale=0.5)
                nc.scalar.activation(TF[:, :], PGs[1][:, :], ACTF.Tanh, scale=0.5)
                nc.scalar.activation(TG[:, :], PGs[2][:, :], ACTF.Tanh)
                nc.scalar.activation(TO[:, :], PGs[3][:, :], ACTF.Tanh, scale=0.5)

                A1 = sp.tile([D, 256], f32, tag="A1")
                A2 = sp.tile([D, 256], f32, tag="A2")
                TC = sp.tile([D, 256], f32, tag="TC")
                nc.vector.scalar_tensor_tensor(
                    A1[:, :], TF[:, :], 1.0, CT[:, :], op0=ALU.add, op1=ALU.mult)
                nc.vector.scalar_tensor_tensor(
                    A2[:, :], TI[:, :], 1.0, TG[:, :], op0=ALU.add, op1=ALU.mult)
                nc.vector.scalar_tensor_tensor(
                    CT[:, :], A1[:, :], 0.5, A2[:, :], op0=ALU.mult, op1=ALU.add)
                nc.scalar.activation(TC[:, :], CT[:, :], ACTF.Tanh, scale=0.5)
                nc.vector.scalar_tensor_tensor(
                    HT65[0:D, :], TO[:, :], 1.0, TC[:, :], op0=ALU.add, op1=ALU.mult)

            # ---- final projection ----
            FIN = pp.tile([1, BC], f32, tag="WO", name="FIN")
            nc.tensor.matmul(FIN[:, :], YW[:, 0:1], HT65[0:D, :], start=True, stop=False)
            nc.tensor.matmul(FIN[:, :], YW[:, 1:2], CT[:, :], start=False, stop=True)
            OUTR = mp.tile([1, BC], f32, tag="OUTR")
            nc.vector.tensor_scalar_add(OUTR[:, :], FIN[:, :], consts["y_b0"])
            nc.sync.dma_start(out_d[:, :], OUTR[:, :])

    return nc


def _make_in_maps(h_encoder, y, consts):
    # host layout prep (no arithmetic): h_enc^T and per-core shards
    ht = np.ascontiguousarray(h_encoder.transpose(2, 0, 1))   # (E,W,B)
    in_maps = []
    for c in range(NCORES):
        slab = ht[:, :, c * BC:(c + 1) * BC]                  # (E,W,256)
        het = np.ascontiguousarray(
            slab.reshape(E, W, 2, 128).transpose(0, 2, 1, 3).reshape(E, 2 * W * 128)
        )
        ys = y[c * BC:(c + 1) * BC, :]                        # (256,W)
        yv = np.ascontiguousarray(
            ys.reshape(2, 128, W).transpose(1, 0, 2).reshape(128, 2 * W)
        )
        in_maps.append({
            "het": het, "yv": yv,
            "uwb": consts["uwb"], "wwhub": consts["wwhub"], "wwc": consts["wwc"],
            "whht": consts["whht"], "w2": consts["w2"], "vb": consts["vb"],
            "yw": consts["yw"], "idn": consts["idn"],
            "ones128": consts["ones128"],
        })
    return in_maps


consts = None  # set inside kernel(); _build_program reads p_w0/y_b0 immediates
TRACE = False       # test harness sets True to capture an NTFF profile
LOOP_REPS = 1       # test harness uses 0/1/2 for differential timing
LAST_RESULT = None  # BassKernelResults of the most recent kernel() call


def kernel(**inputs) -> np.ndarray:
    global consts
    _install_tile_patch()
    from concourse.bass_utils import run_bass_kernel_spmd

    f32 = np.float32
    h_encoder = np.asarray(inputs["h_encoder"], f32)   # (W,B,E)
    y = np.asarray(inputs["y"], f32)                   # (B,W)
    consts = _pack_consts(inputs)
    in_maps = _make_in_maps(h_encoder, y, consts)

    nc = _build_program(consts, loop_reps=LOOP_REPS)
    global LAST_RESULT
    res = run_bass_kernel_spmd(
        nc, in_maps, core_ids=list(range(NCORES)), trace=TRACE
    )
    LAST_RESULT = res
    out = np.concatenate([res.results[c]["out"].reshape(BC) for c in range(NCORES)])
    return out.astype(np.float32)


if __name__ == "__main__":
    import reference as refmod
    ins = {k: np.asarray(v) for k, v in refmod.setup_inputs().items()}
    got = kernel(**ins)
    import jax.numpy as jnp
    exp = np.asarray(refmod.reference(**{k: jnp.asarray(v) for k, v in ins.items()}))
    rel = np.linalg.norm(got - exp) / np.linalg.norm(exp)
    print("rel l2 err:", rel)
